# revision 1
# baseline (speedup 1.0000x reference)
"""EnhancedChannelFilter Trainium2 kernel.

Full inputs in, full outputs out. Internally: pure data-parallel over 8
NeuronCores (4 images each), NCHW layout with channels on SBUF partitions.

Per core, per image (x_img = [256, 3136] as 2 partition-halves):
  1. Packet-loss mask: elementwise mask in NHWC-flat space is constant across
     groups of 16 channels (gcd(368, 256) = 16), so it is shipped as a tiny
     fp8 [8, 2, 3136] tensor per image (0/1 exact in fp8e4) and expanded on
     the PE by a 0/1 select-matrix DoubleRow matmul into PSUM at 0.5
     cycles/row.
  2. x_m = x * mask fused with the SE row-sum via DVE scalar_tensor_tensor
     accum_out (per-n-tile partials + one small reduce).
  3. SE chain (fc1 -> relu -> fc2 -> sigmoid -> +bias -> relu) on PE/ACT,
     producing a per-channel scale; 1/HW and rate*adapt_w - threshold are
     folded into the weights on the host.
  4. det / rec1 / rec2 1x1-conv GEMMs in float32r (full PE rate, fp32
     storage), K on partitions, pixels streamed in 448-wide n-tiles,
     m-halves packed into [128, 1024] PSUM tiles (one bank per half).
  5. sigmoid/relu evictions on ACT, zh = sigmoid(det)*x_m on GpSimd,
     final per-channel scale on DVE tensor_scalar, direct DMA out.
"""

import math

import numpy as np
import ml_dtypes

B, C, H, W = 32, 256, 56, 56
HW = H * W              # 3136
NCORES = 8
BC = B // NCORES        # images per core
NT = 448                # pixels per n-tile
NTILES = HW // NT       # 7
EPC = 1472 // 4         # f32 elements per packet chunk (368)
QG = 16                 # channel-group size: gcd(EPC, C)
UPC = EPC // QG         # 23 channel-group-units per chunk

_CACHE: dict = {}


# ---------------------------------------------------------------------------
# Workaround: this walrus build enforces 1 sync wait per instruction (2 for
# EventSemaphore), but the Tile framework attaches several to its exit drain.
# Splitting extra waits onto dedicated same-engine NOPs placed immediately
# before the instruction is semantically identical.
# ---------------------------------------------------------------------------
def _split_multiwaits(nc, mybir):
    n = 0
    for bb in nc.m.functions[0].blocks:
        lst = bb.instructions
        for inst in list(lst):
            si = inst.sync_info
            if si is None or not si.on_wait:
                continue
            cap = 2 if isinstance(inst, mybir.InstEventSemaphore) else 1
            waits = list(si.on_wait)
            if len(waits) <= cap:
                continue
            eng = nc.engines[inst.engine]
            extra = []
            for wt in waits[:-cap]:
                nop = eng.nop(nofuse=True).ins
                nop.sync_info = mybir.SyncInfo(on_wait=[wt], on_update=[])
                nc.cur_bb.bb.instructions.remove(nop)
                extra.append(nop)
            si.on_wait = waits[-cap:]
            idx = lst.index(inst)
            lst[idx:idx] = extra
            n += 1
    return n


def _build(debug=False, repeat=0, wbf16=False, gemm_bf16=False):
    import concourse.bass as bass
    import concourse.tile as tile
    import concourse.mybir as mybir

    f32 = mybir.dt.float32
    f32r = mybir.dt.float32r
    bf16 = mybir.dt.bfloat16
    fp8 = mybir.dt.float8e4
    DR = mybir.MatmulPerfMode.DoubleRow
    MULT = mybir.AluOpType.mult
    ADD = mybir.AluOpType.add
    BYPASS = mybir.AluOpType.bypass
    SIGMOID = mybir.ActivationFunctionType.Sigmoid
    COPY = mybir.ActivationFunctionType.Copy
    RELU = mybir.ActivationFunctionType.Relu

    nc = bass.Bass("TRN2", target_bir_lowering=False, debug=False)

    x_d = nc.dram_tensor("x", [BC, 2, 128, HW], f32, kind="ExternalInput").ap()
    m16_d = nc.dram_tensor("m16", [BC, 8, 2, HW], fp8, kind="ExternalInput").ap()
    # f32r GEMM weights packed column-wise: 16x[128,128] (det k,m | rec1 k,m |
    # rec2 k,m) then fc1 2x[128,16], then abias 2x[128,1]
    if gemm_bf16:
        wbf16 = True
    adt = bf16 if gemm_bf16 else f32r   # activation dtype on the GEMM path
    wdt = bf16 if wbf16 else f32r
    wblob_d = nc.dram_tensor("wblob", [128, 16 * 128 + 2 * 16 + 2], f32r,
                             kind="ExternalInput").ap()
    if wbf16:
        wgemm_d = nc.dram_tensor("wgemm", [128, 16 * 128], bf16,
                                 kind="ExternalInput").ap()
    # [16, .]-partition smalls: wfc2 2x[16,128] f32 and emat 2x[16,128] bf16
    wsm_d = nc.dram_tensor("wsm", [16, 2 * 128], f32, kind="ExternalInput").ap()
    esm_d = nc.dram_tensor("esm", [8, 2, 2 * 128], fp8, kind="ExternalInput").ap()
    out_d = nc.dram_tensor("out", [BC, 2, 128, HW], f32, kind="ExternalOutput").ap()
    if debug:
        dxm_d = nc.dram_tensor("dxm", [BC, 2, 128, HW], f32, kind="ExternalOutput").ap()
        dsg_d = nc.dram_tensor("dsg", [BC, 2, 128, HW], f32, kind="ExternalOutput").ap()
        dr1_d = nc.dram_tensor("dr1", [BC, 2, 128, HW], f32, kind="ExternalOutput").ap()
        dmc_d = nc.dram_tensor("dmc", [BC, 2, 128, 1], f32, kind="ExternalOutput").ap()
        dy_d = nc.dram_tensor("dy", [BC, 2, 128, 8], f32, kind="ExternalOutput").ap()

    def r(ap):
        return ap.bitcast(f32r)

    with tile.TileContext(nc) as tc:
        with (
            tc.tile_pool(name="consts", bufs=1) as cpool,
            tc.tile_pool(name="xin", bufs=2) as xpool,
            tc.tile_pool(name="xm", bufs=3) as xmpool,
            tc.tile_pool(name="m16", bufs=2) as m16pool,
            tc.tile_pool(name="sg", bufs=3) as sgpool,
            tc.tile_pool(name="zh", bufs=3) as zhpool,
            tc.tile_pool(name="r1", bufs=3) as r1pool,
            tc.tile_pool(name="osb", bufs=2) as opool,
            tc.tile_pool(name="ysum", bufs=2) as ypool,
            tc.tile_pool(name="mch", bufs=4) as mcpool,
            tc.tile_pool(name="sesb", bufs=2) as sepool,
            tc.tile_pool(name="mp", bufs=2, space="PSUM") as mppool,
            tc.tile_pool(name="dp", bufs=1, space="PSUM") as dppool,
            tc.tile_pool(name="r1p", bufs=1, space="PSUM") as r1ppool,
            tc.tile_pool(name="r2p", bufs=2, space="PSUM") as r2ppool,
        ):
            # ---- constants into SBUF (3 DMAs) ----
            wblob = cpool.tile([128, 16 * 128 + 2 * 16 + 2], f32r, name="wblob",
                               tag="wblob")
            wsm = cpool.tile([16, 2 * 128], f32, name="wsm", tag="wsm")
            esm = cpool.tile([8, 2, 2 * 128], fp8, name="esm", tag="esm")
            nc.sync.dma_start(esm[:], esm_d[:])
            nc.sync.dma_start(wsm[:], wsm_d[:])

            if wbf16:
                wgemm = cpool.tile([128, 16 * 128], bf16, name="wgemm", tag="wgemm")
                nc.sync.dma_start(wgemm[:], wgemm_d[:])
            if repeat:
                nc.sync.dma_start(wblob[:], wblob_d[:])

            def _wcol(i):
                if wbf16:
                    return wgemm[:, i * 128:(i + 1) * 128]
                return wblob[:, i * 128:(i + 1) * 128]

            wdet_sb = [[_wcol(k * 2 + m) for m in range(2)] for k in range(2)]
            wrec1_sb = [[_wcol(4 + k * 2 + m) for m in range(2)] for k in range(4)]
            wrec2_sb = [[_wcol(12 + k * 2 + m) for m in range(2)] for k in range(2)]
            wfc1_sb = [wblob[:, 2048 + k * 16:2048 + (k + 1) * 16].bitcast(f32)
                       for k in range(2)]
            abias_sb = [wblob[:, 2080 + h:2081 + h].bitcast(f32) for h in range(2)]
            wfc2_sb = [wsm[:, m * 128:(m + 1) * 128] for m in range(2)]
            emat_sb = [esm[:, :, h * 128:(h + 1) * 128] for h in range(2)]

            # ---- per-image pipeline, software-pipelined emission:
            # phase1(b) = DMAs + mask-expand + masked-mul/row-sum + SE chain
            # phase2(b) = det/rec GEMM pipeline + final scale + out DMA
            # Emitting phase1(b+1) before phase2(b) raises its scheduler
            # priority so DVE/PE prefetch the next image's prep work.
            st = {}

            def phase1(b):
                m16_sb = m16pool.tile([8, 2, HW], fp8, name=f"m16_b{b}", tag="m16")
                nc.sync.dma_start(m16_sb[:], m16_d[b])
                x_sb = [xpool.tile([128, HW], f32, name=f"x_b{b}h{h}", tag=f"x{h}")
                        for h in range(2)]
                for h in range(2):
                    nc.sync.dma_start(x_sb[h][:, 0:HW // 2], x_d[b, h, :, 0:HW // 2])
                if b == 0 and not repeat:
                    nc.sync.dma_start(wblob[:], wblob_d[:])
                for h in range(2):
                    nc.sync.dma_start(x_sb[h][:, HW // 2:HW], x_d[b, h, :, HW // 2:HW])

                xm_sb = [xmpool.tile([128, HW], adt, name=f"xm_b{b}h{h}", tag=f"xm{h}")
                         for h in range(2)]
                ysum = [ypool.tile([128, 8], f32, name=f"ysum_b{b}h{h}", tag=f"ysum{h}")
                        for h in range(2)]

                for j in range(NTILES):
                    n0 = j * NT
                    for h in range(2):
                        mp = mppool.tile([128, NT], f32, name=f"mp_b{b}h{h}j{j}", tag="mp")
                        nc.tensor.matmul(
                            mp[:], emat_sb[h], m16_sb[:, :, n0:n0 + NT],
                            start=True, stop=True, perf_mode=DR,
                        )
                        nc.vector.scalar_tensor_tensor(
                            out=xm_sb[h][:, n0:n0 + NT],
                            in0=x_sb[h][:, n0:n0 + NT],
                            scalar=0.0,
                            in1=mp[:],
                            op0=BYPASS,
                            op1=MULT,
                            accum_out=ysum[h][:, j:j + 1],
                        )
                if debug:
                    for h in range(2):
                        nc.sync.dma_start(dxm_d[b, h], xm_sb[h][:].bitcast(f32))

                # SE chain -> per-channel output scale mc[h]
                for h in range(2):
                    nc.vector.reduce_sum(ysum[h][:, 7:8], ysum[h][:, 0:NTILES],
                                         axis=mybir.AxisListType.X)
                fc1p = r2ppool.tile([16, 1], f32, name=f"fc1p_b{b}", tag="r2p")
                nc.tensor.matmul(fc1p[:], wfc1_sb[0][:], ysum[0][:, 7:8],
                                 start=True, stop=False)
                nc.tensor.matmul(fc1p[:], wfc1_sb[1][:], ysum[1][:, 7:8],
                                 start=False, stop=True)
                h1 = sepool.tile([16, 1], f32, name=f"h1_b{b}", tag="h1")
                nc.scalar.activation(h1[:], fc1p[:], RELU)
                mc = []
                for h in range(2):
                    scp = r2ppool.tile([128, 1], f32, name=f"scp_b{b}h{h}", tag="r2p")
                    nc.tensor.matmul(scp[:], wfc2_sb[h][:], h1[:],
                                     start=True, stop=True)
                    ssb = sepool.tile([128, 1], f32, name=f"ssb_b{b}h{h}", tag="ssb")
                    nc.scalar.activation(ssb[:], scp[:], SIGMOID)
                    mch = mcpool.tile([128, 1], f32, name=f"mc_b{b}h{h}", tag="mc")
                    nc.scalar.activation(mch[:], ssb[:], RELU, bias=abias_sb[h][:])
                    mc.append(mch)
                if debug:
                    for h in range(2):
                        nc.sync.dma_start(dy_d[b, h], ysum[h][:])
                        nc.sync.dma_start(dmc_d[b, h], mc[h][:])
                st[b] = (xm_sb, mc)

            def phase2(b, jlist=None):
                if jlist is None:
                    jlist = range(NTILES)
                xm_sb, mc = st[b]
                ot = {}
                for j in jlist:
                    n0 = j * NT
                    xmn = [xm_sb[h][:, n0:n0 + NT] for h in range(2)]

                    dp = dppool.tile([128, 1024], f32, name=f"dp_b{b}j{j}", tag="dp")
                    for m in range(2):
                        for k in range(2):
                            nc.tensor.matmul(
                                dp[:, m * 512:m * 512 + NT],
                                wdet_sb[k][m][:], xmn[k],
                                start=(k == 0), stop=(k == 1),
                            )
                    sg = sgpool.tile([128, 2 * NT], adt if gemm_bf16 else f32,
                                     name=f"sg_b{b}j{j}", tag="sg")
                    nc.scalar.activation(
                        sg.rearrange("p (m w) -> p m w", w=NT),
                        dp.rearrange("p (m w) -> p m w", w=512)[:, :, 0:NT],
                        SIGMOID,
                    )
                    if debug:
                        for m in range(2):
                            nc.sync.dma_start(dsg_d[b, m, :, n0:n0 + NT],
                                              sg[:, m * NT:(m + 1) * NT])
                    zh = []
                    for h in range(2):
                        z = zhpool.tile([128, NT], adt, name=f"zh_b{b}h{h}j{j}", tag=f"zh{h}")
                        eng = nc.vector if h == 0 else nc.gpsimd
                        eng.tensor_tensor(
                            z[:], sg[:, h * NT:(h + 1) * NT], xmn[h], MULT,
                        )
                        zh.append(z)

                    r1p = r1ppool.tile([128, 1024], f32, name=f"r1p_b{b}j{j}", tag="r1p")
                    kts = [xmn[0], xmn[1], zh[0][:], zh[1][:]]
                    for m in range(2):
                        for k in range(4):
                            nc.tensor.matmul(
                                r1p[:, m * 512:m * 512 + NT],
                                wrec1_sb[k][m][:], kts[k],
                                start=(k == 0), stop=(k == 3),
                            )
                    r1sb = r1pool.tile([128, 2 * NT], adt, name=f"r1_b{b}j{j}", tag="r1")
                    nc.scalar.activation(
                        r1sb.rearrange("p (m w) -> p m w", w=NT),
                        r1p.rearrange("p (m w) -> p m w", w=512)[:, :, 0:NT],
                        RELU,
                    )
                    if debug:
                        for m in range(2):
                            nc.sync.dma_start(dr1_d[b, m, :, n0:n0 + NT],
                                              r1sb[:, m * NT:(m + 1) * NT].bitcast(f32))

                    for m in range(2):
                        r2p = r2ppool.tile([128, NT], f32, name=f"r2p_b{b}m{m}j{j}",
                                           tag="r2p")
                        for k in range(2):
                            nc.tensor.matmul(
                                r2p[:],
                                wrec2_sb[k][m][:],
                                r1sb[:, k * NT:(k + 1) * NT],
                                start=(k == 0), stop=(k == 1),
                            )
                        # pair n-tiles into one [128, 2*NT] out tile per m:
                        # one DMA per (m, n-pair) instead of per (m, n)
                        if j % 2 == 0:
                            ot[m] = opool.tile([128, 2 * NT], f32,
                                               name=f"o_b{b}m{m}j{j}", tag=f"o{m}")
                        o = ot[m][:, (j % 2) * NT:(j % 2 + 1) * NT]
                        if (j + m) % 2 == 0:
                            nc.vector.tensor_scalar_mul(o, r2p[:], mc[m][:])
                        else:
                            nc.scalar.activation(o, r2p[:], COPY, scale=mc[m][:])
                        if j % 2 == 1 or j == NTILES - 1:
                            w = (j % 2 + 1) * NT
                            nc.sync.dma_start(
                                out_d[b, m, :, n0 - (j % 2) * NT:n0 + NT],
                                ot[m][:, 0:w])

            import contextlib as _ctxlib
            rep_cm = (tc.For_i(0, repeat, 1,
                               hint_engines=(mybir.EngineType.PE,
                                             mybir.EngineType.DVE,
                                             mybir.EngineType.Activation,
                                             mybir.EngineType.SP,
                                             mybir.EngineType.Pool))
                      if repeat else _ctxlib.nullcontext())
            with rep_cm:
                for b in range(BC):
                    phase1(b)
                    phase2(b)
                    del st[b]

    _split_multiwaits(nc, mybir)
    return nc


def _jax_perm_cpu(num_chunks: int) -> np.ndarray:
    """jax.random.permutation(key(1234), num_chunks) on the CPU backend.

    Run in a JAX_PLATFORMS=cpu subprocess: in this process jax may be bound
    to an accelerator backend that cannot lower the shuffle's sort op.
    """
    import os
    import subprocess
    import sys
    import tempfile

    import jax

    sp = os.path.dirname(os.path.dirname(jax.__file__))
    code = (
        "import sys, numpy as np, jax\n"
        f"perm = np.asarray(jax.random.permutation(jax.random.key(1234), {num_chunks}))\n"
        "np.save(sys.argv[1], perm)\n"
    )
    with tempfile.TemporaryDirectory() as td:
        path = os.path.join(td, "perm.npy")
        env = dict(os.environ, JAX_PLATFORMS="cpu", PYTHONPATH=sp)
        env.pop("TRN_TERMINAL_POOL_IPS", None)
        subprocess.run([sys.executable, "-c", code, path], env=env, check=True)
        return np.load(path)


def _mask16(rate: int) -> np.ndarray:
    """Per-image [16, HW] bf16 keep-mask in channel-group space."""
    n = B * C * HW
    num_chunks = math.ceil(n * 4 / 1472)
    num_lossy = int(math.ceil(num_chunks * (rate / 100)))
    perm = _jax_perm_cpu(num_chunks)
    keep = np.ones((num_chunks,), np.float32)
    if num_lossy > 0:
        keep[perm[:num_lossy]] = 0.0
    bg = np.arange(B, dtype=np.int64)
    qq = np.arange(QG, dtype=np.int64)
    pp = np.arange(HW, dtype=np.int64)
    u = (bg[:, None, None] * HW + pp[None, None, :]) * QG + qq[None, :, None]
    return keep[u // UPC].astype(ml_dtypes.float8_e4m3)


def _prep_in_maps(inputs):
    x = np.ascontiguousarray(np.asarray(inputs["x"], dtype=np.float32))
    rate = int(np.asarray(inputs["Packet_Loss_Rate"]))
    fc1 = np.asarray(inputs["fc1_w"], dtype=np.float32)
    fc2 = np.asarray(inputs["fc2_w"], dtype=np.float32)
    thr = float(np.asarray(inputs["threshold"], dtype=np.float32).reshape(-1)[0])
    detw = np.asarray(inputs["detect_w"], dtype=np.float32)
    r1w = np.asarray(inputs["rec1_w"], dtype=np.float32)
    r2w = np.asarray(inputs["rec2_w"], dtype=np.float32)
    aw = np.asarray(inputs["adapt_w"], dtype=np.float32)

    blob = np.zeros((128, 16 * 128 + 2 * 16 + 2), np.float32)

    def put(i, tile):
        blob[:, i * 128:(i + 1) * 128] = tile

    detT, r1T, r2T = detw.T, r1w.T, r2w.T
    for k in range(2):
        for m in range(2):
            put(k * 2 + m, detT[k * 128:(k + 1) * 128, m * 128:(m + 1) * 128])
    for k in range(4):
        for m in range(2):
            put(4 + k * 2 + m, r1T[k * 128:(k + 1) * 128, m * 128:(m + 1) * 128])
    for k in range(2):
        for m in range(2):
            put(12 + k * 2 + m, r2T[k * 128:(k + 1) * 128, m * 128:(m + 1) * 128])
    fc1T = fc1.T / HW                                  # [256, 16]
    blob[:, 2048:2064] = fc1T[0:128]
    blob[:, 2064:2080] = fc1T[128:256]
    ab = (rate * aw[:, 0] - thr).astype(np.float32)
    blob[:, 2080] = ab[0:128]
    blob[:, 2081] = ab[128:256]

    wsm = np.ascontiguousarray(fc2.T.astype(np.float32))          # [16, 256]
    emat = np.zeros((16, 256), np.float32)
    for h in range(2):
        for cc in range(128):
            emat[(h * 128 + cc) // QG, h * 128 + cc] = 1.0
    # DoubleRow fp8 layout: K-rows (2q, 2q+1) packed per physical partition
    esm = emat.astype(ml_dtypes.float8_e4m3).reshape(8, 2, 256)
    m16 = _mask16(rate).reshape(B, 8, 2, HW)

    xr = x.reshape(B, 2, 128, HW)
    in_maps = []
    for c in range(NCORES):
        in_maps.append({
            "x": xr[c * BC:(c + 1) * BC],
            "m16": m16[c * BC:(c + 1) * BC],
            "wblob": blob, "wsm": wsm, "esm": esm,
        })
    return in_maps


def _add_wgemm(in_maps):
    wg = in_maps[0]["wblob"][:, 0:2048].astype(ml_dtypes.bfloat16)
    for m in in_maps:
        m["wgemm"] = wg
    return in_maps


def kernel(**inputs) -> np.ndarray:
    from concourse.bass_utils import run_bass_kernel_spmd

    in_maps = _prep_in_maps(inputs)
    if "nc" not in _CACHE:
        _CACHE["nc"] = _build()
    nc = _CACHE["nc"]
    last_err = None
    for _attempt in range(3):
        try:
            res = run_bass_kernel_spmd(nc, in_maps, core_ids=list(range(NCORES)))
            break
        except Exception as e:  # transient axon/device hiccups: retry
            last_err = e
    else:
        raise last_err
    out = np.stack([res.results[c]["out"] for c in range(NCORES)], axis=0)
    return out.reshape(B, C, H, W).astype(np.float32)

